# revision 29
# baseline (speedup 1.0000x reference)
"""Causal self-attention Trainium2 kernel (B=4, T=2048, E=1024, H=16, D=64).

Sharding: 8 cores = batch(4) x head-group(2). Each core computes the full
attention for 8 heads of one batch element plus its half of the output
projection; the host sums the two out-proj partials per batch element.

Dataflow (per core, all matmul operands fp16, PSUM fp32):
  - Host pre-transposes x and the weights so contraction dims land on
    partitions: xT [E,T], wqkvT [E,1536], woT [512,E], all fp16.
  - Projection produces Q^T/K^T in [d,T] layout (head pairs packed into 128
    partitions) and V in natural [T,d] layout with an interleaved ones
    column per head parity: even heads [d|1], odd heads [1|d] so the PV
    output of the odd head can land on PSUM partitions 63..127 (l on 63,
    y on 64..127) while the even head lands on 0..64 (y 0..63, l 64).
  - Attention per (query tile, head pair): S^T chunks [128kv, <=512q],
    causally trimmed on diagonal chunks -> exp on ScalarE (scale=1/sqrt(D),
    fp16 out) -> triangular affine_select on GpSimd for the single
    diagonal 128x128 sub-block -> yT accumulation with lhsT = V_aug.
  - Softmax normalize without ScalarE: copy the denominator rows to SBUF
    (partition-aligned), GpSimd partition_broadcast, one DVE divide per
    head half writing fp16 yt.
  - Out-projection fused per query tile: natural-layout [T,E] partial via
    lhsT = yT slices.
"""

import os
import numpy as np

import concourse.bass as bass
import concourse.bacc as bacc
import concourse.mybir as mybir
import concourse.tile as tile
from concourse import bass_utils

f32 = mybir.dt.float32
f16 = mybir.dt.float16
FP = mybir.dt.float32  # psum dtype

P = 128
B, T, E = 4, 2048, 1024
H, D = 16, 64
HPC = H // 2            # heads per core = 8
NE = E // P             # 8 e-chunks
NTT = T // P            # 16 kv chunks
NQ = T // 512           # 4 query tiles of 512
SCALE = 1.0 / np.sqrt(D)

Exp = mybir.ActivationFunctionType.Exp
MULT = mybir.AluOpType.mult
DIV = mybir.AluOpType.divide
IS_GE = mybir.AluOpType.is_ge

_CACHE = {}


def build(reps=1, dbg=False, **opts):
    nc = bacc.Bacc("TRN2", target_bir_lowering=False, debug=False, num_devices=8)

    # host-prepacked [partition, ...] layouts so every DMA line is >=2KB
    xP_d = nc.dram_tensor("xP", [P, NE, T], f16, kind="ExternalInput")
    wqkP_d = nc.dram_tensor("wqkP", [P, 8, NE, P], f16, kind="ExternalInput")
    wvP_d = nc.dram_tensor("wvP", [P, NE, 512], f16, kind="ExternalInput")
    woP_d = nc.dram_tensor("woP", [P, 4, E], f16, kind="ExternalInput")
    out_d = nc.dram_tensor("out", [T, E], f32, kind="ExternalOutput")

    dbg_d = None
    if dbg:
        dbg_d = {
            "dbg_qk": nc.dram_tensor("dbg_qk", [P, 8, T], f16,
                                     kind="ExternalOutput"),
            "dbg_v": nc.dram_tensor("dbg_v", [P, NTT, HPC, D + 1], f16,
                                    kind="ExternalOutput"),
            "dbg_l": nc.dram_tensor("dbg_l", [NQ, 4, 1024], f32,
                                    kind="ExternalOutput"),
            "dbg_rc": nc.dram_tensor("dbg_rc", [NQ, 4, 1024], f32,
                                     kind="ExternalOutput"),
            "dbg_bc": nc.dram_tensor("dbg_bc", [NQ, 4, 64, 1024], f32,
                                     kind="ExternalOutput"),
            "dbg_yt": nc.dram_tensor("dbg_yt", [P, 4, T], f16,
                                     kind="ExternalOutput"),
        }

    with tile.TileContext(nc) as tc:
        for rep in range(reps):
            build_body(tc, xP_d, wqkP_d, wvP_d, woP_d, out_d, rep, dbg_d=dbg_d, **opts)
    nc.compile()
    return nc


def build_body(tc, xP_d, wqkP_d, wvP_d, woP_d, out_d, rep=0,
               ptp_bufs=6, pss_bufs=2, psy_bufs=2, dbg_d=None):
    nc = tc.nc

    from contextlib import ExitStack
    with ExitStack() as top:
        per = top.enter_context(tc.tile_pool(name="per", bufs=1))
        wpool = top.enter_context(tc.tile_pool(name="wpool", bufs=2))
        xpool = top.enter_context(tc.tile_pool(name="xpool", bufs=1))
        ptp = top.enter_context(tc.tile_pool(name="ptp", bufs=ptp_bufs))
        nrm = top.enter_context(tc.tile_pool(name="nrm", bufs=2))
        ost = top.enter_context(tc.tile_pool(name="ost", bufs=2))
        # PSUM budget (8 banks): psp 2 (proj + out-proj chains, one tag),
        # pss 2x2 (S chunks), psy 2x1 (PV accumulators)
        psp = top.enter_context(tc.tile_pool(name="psp", bufs=2, space="PSUM"))
        pss = top.enter_context(
            tc.tile_pool(name="pss", bufs=pss_bufs, space="PSUM"))
        psy = top.enter_context(
            tc.tile_pool(name="psy", bufs=psy_bufs, space="PSUM"))

        qk_sb = per.tile([P, 8, T], f16)             # chunks 0-3: Q^T, 4-7: K^T
        v_sb = per.tile([P, NTT, HPC, D + 1], f16)   # [kv_p, kv_chunk, head, d|1]
        yt_sb = per.tile([P, 4, T], f16)             # [f%128, f//128, q]
        wo_sb = per.tile([P, 4, 1024], f16)

        # ones column for the softmax denominator ride-along (once, up front)
        nc.vector.memset(v_sb[:, :, :, D:D + 1], 1.0)

        # spread DMA trigger issue (~0.6us each on an engine queue) across
        # the otherwise-idle engines, and keep transfers small enough that
        # the first projection chains start while later slices stream in
        x_all = xpool.tile([P, NE, T], f16, tag="x")
        wv_sb = wpool.tile([P, NE, 512], f16, tag="wv", bufs=1)
        wqk_all = wpool.tile([P, 8, NE, P], f16, tag="wq", bufs=1)
        issuers = [nc.sync, nc.scalar, nc.gpsimd]
        nd = 0

        def dma(dst, srcv):
            nonlocal nd
            issuers[nd % len(issuers)].dma_start(dst, srcv)
            nd += 1

        for th in range(4):
            tsl = slice(th * 512, (th + 1) * 512)
            for e in range(NE):
                dma(x_all[:, e, tsl], xP_d[:, e, tsl])
            if th == 0:
                for e in range(NE):
                    dma(wv_sb[:, e, :], wvP_d[:, e, :])
            if th == 1:
                for ft in range(8):
                    dma(wqk_all[:, ft], wqkP_d[:, ft])
        for o in range(4):
            dma(wo_sb[:, o, :], woP_d[:, o, :])

        def proj_quarter(th):
            tsl = slice(th * 512, (th + 1) * 512)
            # V projection for this quarter: natural layout [T, 512]
            for tti in range(4):
                tt = th * 4 + tti
                ps = psp.tile([P, 512], FP, tag="pp")
                for e in range(NE):
                    nc.tensor.matmul(
                        ps,
                        lhsT=x_all[:, e, tt * P:(tt + 1) * P],
                        rhs=wv_sb[:, e, :],
                        start=(e == 0), stop=(e == NE - 1))
                nc.vector.tensor_copy(
                    v_sb[:, tt, :, 0:D],
                    ps.rearrange("p (h d) -> p h d", h=HPC))

            # QK^T projection for this T-quarter: [f, T] layout
            for ft in (0, 4, 1, 5, 2, 6, 3, 7):
                ps = psp.tile([P, 512], FP, tag="pp")
                for e in range(NE):
                    nc.tensor.matmul(
                        ps,
                        lhsT=wqk_all[:, ft, e, :],
                        rhs=x_all[:, e, tsl],
                        start=(e == 0), stop=(e == NE - 1))
                nc.vector.tensor_copy(qk_sb[:, ft, tsl], ps)

        def att_core(j):
            jsl = slice(j * 512, (j + 1) * 512)
            for c in range(4):  # head pair (2c, 2c+1)
                nkv = 4 * j + 4
                yps = [psy.tile([P, 512], FP, tag="y",
                                name=f"yps{rep}_{c}_{j}_{k}")[0:65]
                       for k in range(2)]

                def emit_pv(i, pv):
                    # the stop matmul must cover the full accumulation width
                    # (HW PSUM group-close semantics), so the final chunk's
                    # PV runs untrimmed; its dead ptt region was zeroed.
                    off = i - 4 * j
                    pq0 = P * off if (off > 0 and i < nkv - 1) else 0
                    for hh in range(2):
                        nc.tensor.matmul(
                            yps[hh][:, pq0:512],
                            lhsT=v_sb[:, i, 2 * c + hh, :],
                            rhs=pv[:, hh, pq0:512],
                            start=(i == 0), stop=(i == nkv - 1),
                            skip_group_check=True)

                # PV trails S by one chunk so the PE does not sit on exp(i)
                prev_pv = None
                for i in range(nkv):
                    off = i - 4 * j
                    q0 = P * off if off > 0 else 0
                    spt = pss.tile([P, 1024], FP, tag="s")
                    for hh in range(2):
                        h0 = hh * 512
                        nc.tensor.matmul(
                            spt[:, h0 + q0:h0 + 512],
                            lhsT=qk_sb[64 * hh:64 * hh + 64, 4 + c,
                                       i * P:(i + 1) * P],
                            rhs=qk_sb[64 * hh:64 * hh + 64, c,
                                      j * 512 + q0:(j + 1) * 512],
                            start=True, stop=True)
                    ptt = ptp.tile([P, 1024], f16, tag="pt")
                    pv = ptt.rearrange("p (h q) -> p h q", h=2)
                    sv = spt.rearrange("p (h q) -> p h q", h=2)
                    if off < 0:
                        nc.scalar.activation(ptt, spt, Exp, scale=float(SCALE))
                    else:
                        # diagonal chunk: exp live columns only, then causal
                        # select of the 128-wide triangle block
                        nc.scalar.activation(pv[:, :, q0:512], sv[:, :, q0:512],
                                             Exp, scale=float(SCALE))
                        if i == nkv - 1:
                            # final chunk feeds the full-width stop matmul:
                            # zero the dead cols (write-only, no stale read)
                            nc.gpsimd.memset(pv[:, :, 0:q0], 0.0)
                        nc.gpsimd.affine_select(
                            pv[:, :, q0:q0 + P], pv[:, :, q0:q0 + P],
                            pattern=[[0, 2], [1, P]],
                            compare_op=IS_GE,
                            fill=0.0, base=0, channel_multiplier=-1)
                    if prev_pv is not None:
                        emit_pv(*prev_pv)
                    prev_pv = (i, pv)
                emit_pv(*prev_pv)

                # normalize: 1/l on DVE (approx-fast), GpSimd broadcast, MULT.
                # Single-row ops run at partition 0 (recip/broadcast misbehave
                # on HW at partition offset 64); ScalarE does the
                # cross-partition PSUM p64 -> SBUF p0 hop.
                lsb = nrm.tile([1, 1024], f32, tag="lg")
                rcb = nrm.tile([1, 1024], f32, tag="rc")
                bc = nrm.tile([P, 1024], f32, tag="bc")
                for hh in range(2):
                    hsl = slice(hh * 512, (hh + 1) * 512)
                    nc.scalar.copy(lsb[0:1, hsl], yps[hh][64:65, :])
                    nc.vector.reciprocal_approx_fast(
                        rcb[0:1, hsl], lsb[0:1, hsl])
                    nc.gpsimd.partition_broadcast(
                        bc[0:64, hsl], rcb[0:1, hsl])
                nc.vector.tensor_tensor(
                    yt_sb[0:64, c, jsl], yps[0][0:64, :], bc[0:64, 0:512], MULT)
                tmp = nrm.tile([64, 512], f16, tag="tmp")
                nc.vector.tensor_tensor(
                    tmp, yps[1][0:64, :], bc[0:64, 512:1024], MULT)
                nc.sync.dma_start(yt_sb[64:128, c, jsl], tmp)
                if dbg_d is not None:
                    nc.sync.dma_start(dbg_d["dbg_l"][j, c, :], lsb[0:1, :])
                    nc.sync.dma_start(dbg_d["dbg_rc"][j, c, :], rcb[0:1, :])
                    nc.sync.dma_start(dbg_d["dbg_bc"][j, c, :, :], bc[0:64, :])

        def out_proj(j):
            for tti in range(4):
                tt = 4 * j + tti
                st = ost.tile([P, 1024], f32, tag="st")
                for half in range(2):
                    ps = psp.tile([P, 512], FP, tag="pp")
                    for c2 in range(4):
                        nc.tensor.matmul(
                            ps,
                            lhsT=yt_sb[:, c2, tt * P:(tt + 1) * P],
                            rhs=wo_sb[:, c2, half * 512:(half + 1) * 512],
                            start=(c2 == 0), stop=(c2 == 3))
                    nc.vector.tensor_copy(st[:, half * 512:(half + 1) * 512], ps)
                nc.sync.dma_start(out_d[tt * P:(tt + 1) * P, :], st)

        # Emission (= scheduler priority) order: attention j0/j1 interleaves
        # with projection of the second T-half; out-projections are deferred
        # so their PSUM slots never gate projection, and they act as PE
        # filler during the Scalar-bound later attention blocks.
        proj_quarter(0)
        att_core(0)
        proj_quarter(1)
        att_core(1)
        proj_quarter(2)
        att_core(2)
        proj_quarter(3)
        if dbg_d is not None:
            nc.sync.dma_start(dbg_d["dbg_qk"][:, :, :], qk_sb)
            nc.sync.dma_start(dbg_d["dbg_v"][:, :, :, :], v_sb)
        out_proj(0)
        out_proj(1)
        out_proj(2)
        att_core(3)
        out_proj(3)
        if dbg_d is not None:
            nc.sync.dma_start(dbg_d["dbg_yt"][:, :, :], yt_sb)


def _shard_inputs(x, w_qkv, w_out):
    in_maps = []
    for core in range(8):
        b, hg = core // 2, core % 2
        sl = slice(hg * 512, (hg + 1) * 512)
        wq = w_qkv[0:1024][sl]
        wk = w_qkv[1024:2048][sl]
        wv = w_qkv[2048:3072][sl]
        # [partition, ...] prepack so every DMA line is large and contiguous
        qkT = np.concatenate([wq, wk], axis=0).T.astype(np.float16)  # [E, 1024]
        wqkP = qkT.reshape(NE, P, 8, P).transpose(1, 2, 0, 3)  # [p, ft, e, f]
        wvP = wv.T.astype(np.float16).reshape(NE, P, 512).transpose(1, 0, 2)
        woP = w_out[:, sl].T.astype(np.float16).reshape(4, P, E).transpose(1, 0, 2)
        xP = x[b].T.astype(np.float16).reshape(NE, P, T).transpose(1, 0, 2)
        in_maps.append({
            "xP": np.ascontiguousarray(xP),
            "wqkP": np.ascontiguousarray(wqkP),
            "wvP": np.ascontiguousarray(wvP),
            "woP": np.ascontiguousarray(woP),
        })
    return in_maps


def kernel(x, w_qkv, w_out, _trace=False):
    x = np.asarray(x, dtype=np.float32)
    w_qkv = np.asarray(w_qkv, dtype=np.float32)
    w_out = np.asarray(w_out, dtype=np.float32)

    if "nc" not in _CACHE:
        _CACHE["nc"] = build()
    nc = _CACHE["nc"]

    in_maps = _shard_inputs(x, w_qkv, w_out)
    res = bass_utils.run_bass_kernel_spmd(
        nc, in_maps, core_ids=list(range(8)), trace=_trace)
    kernel.last_result = res

    out = np.empty((B, T, E), dtype=np.float32)
    for b in range(B):
        out[b] = res.results[2 * b]["out"] + res.results[2 * b + 1]["out"]
    return out


# revision 30
# speedup vs baseline: 1.0146x; 1.0146x over previous
"""Causal self-attention Trainium2 kernel (B=4, T=2048, E=1024, H=16, D=64).

Sharding: 8 cores = batch(4) x head-group(2). Each core computes the full
attention for 8 heads of one batch element plus its half of the output
projection; the host sums the two out-proj partials per batch element.

Dataflow (per core, all matmul operands fp16, PSUM fp32):
  - Host pre-transposes x and the weights so contraction dims land on
    partitions: xT [E,T], wqkvT [E,1536], woT [512,E], all fp16.
  - Projection produces Q^T/K^T in [d,T] layout (head pairs packed into 128
    partitions) and V in natural [T,d] layout with an interleaved ones
    column per head parity: even heads [d|1], odd heads [1|d] so the PV
    output of the odd head can land on PSUM partitions 63..127 (l on 63,
    y on 64..127) while the even head lands on 0..64 (y 0..63, l 64).
  - Attention per (query tile, head pair): S^T chunks [128kv, <=512q],
    causally trimmed on diagonal chunks -> exp on ScalarE (scale=1/sqrt(D),
    fp16 out) -> triangular affine_select on GpSimd for the single
    diagonal 128x128 sub-block -> yT accumulation with lhsT = V_aug.
  - Softmax normalize without ScalarE: copy the denominator rows to SBUF
    (partition-aligned), GpSimd partition_broadcast, one DVE divide per
    head half writing fp16 yt.
  - Out-projection fused per query tile: natural-layout [T,E] partial via
    lhsT = yT slices.
"""

import os
import numpy as np

import concourse.bass as bass
import concourse.bacc as bacc
import concourse.mybir as mybir
import concourse.tile as tile
from concourse import bass_utils

f32 = mybir.dt.float32
f16 = mybir.dt.float16
FP = mybir.dt.float32  # psum dtype

P = 128
B, T, E = 4, 2048, 1024
H, D = 16, 64
HPC = H // 2            # heads per core = 8
NE = E // P             # 8 e-chunks
NTT = T // P            # 16 kv chunks
NQ = T // 512           # 4 query tiles of 512
SCALE = 1.0 / np.sqrt(D)

Exp = mybir.ActivationFunctionType.Exp
MULT = mybir.AluOpType.mult
DIV = mybir.AluOpType.divide
IS_GE = mybir.AluOpType.is_ge

_CACHE = {}


def build(reps=1, dbg=False, **opts):
    nc = bacc.Bacc("TRN2", target_bir_lowering=False, debug=False, num_devices=8)

    # host-prepacked [partition, ...] layouts so every DMA line is >=2KB
    xP_d = nc.dram_tensor("xP", [P, NE, T], f16, kind="ExternalInput")
    wqkP_d = nc.dram_tensor("wqkP", [P, 8, NE, P], f16, kind="ExternalInput")
    wvP_d = nc.dram_tensor("wvP", [P, NE, 512], f16, kind="ExternalInput")
    woP_d = nc.dram_tensor("woP", [P, 4, E], f16, kind="ExternalInput")
    out_d = nc.dram_tensor("out", [T, E], f32, kind="ExternalOutput")

    dbg_d = None
    if dbg:
        dbg_d = {
            "dbg_qk": nc.dram_tensor("dbg_qk", [P, 8, T], f16,
                                     kind="ExternalOutput"),
            "dbg_v": nc.dram_tensor("dbg_v", [P, NTT, HPC, D + 1], f16,
                                    kind="ExternalOutput"),
            "dbg_l": nc.dram_tensor("dbg_l", [NQ, 4, 1024], f32,
                                    kind="ExternalOutput"),
            "dbg_rc": nc.dram_tensor("dbg_rc", [NQ, 4, 1024], f32,
                                     kind="ExternalOutput"),
            "dbg_bc": nc.dram_tensor("dbg_bc", [NQ, 4, 64, 1024], f32,
                                     kind="ExternalOutput"),
            "dbg_yt": nc.dram_tensor("dbg_yt", [P, 4, T], f16,
                                     kind="ExternalOutput"),
        }

    with tile.TileContext(nc) as tc:
        for rep in range(reps):
            build_body(tc, xP_d, wqkP_d, wvP_d, woP_d, out_d, rep, dbg_d=dbg_d, **opts)
    nc.compile()
    return nc


def build_body(tc, xP_d, wqkP_d, wvP_d, woP_d, out_d, rep=0,
               ptp_bufs=6, pss_bufs=2, psy_bufs=2, dbg_d=None):
    nc = tc.nc

    from contextlib import ExitStack
    with ExitStack() as top:
        per = top.enter_context(tc.tile_pool(name="per", bufs=1))
        wpool = top.enter_context(tc.tile_pool(name="wpool", bufs=2))
        xpool = top.enter_context(tc.tile_pool(name="xpool", bufs=1))
        ptp = top.enter_context(tc.tile_pool(name="ptp", bufs=ptp_bufs))
        nrm = top.enter_context(tc.tile_pool(name="nrm", bufs=2))
        ost = top.enter_context(tc.tile_pool(name="ost", bufs=2))
        # PSUM budget (8 banks): psp 2 (proj + out-proj chains, one tag),
        # pss 2x2 (S chunks), psy 2x1 (PV accumulators)
        psp = top.enter_context(tc.tile_pool(name="psp", bufs=2, space="PSUM"))
        pss = top.enter_context(
            tc.tile_pool(name="pss", bufs=pss_bufs, space="PSUM"))
        psy = top.enter_context(
            tc.tile_pool(name="psy", bufs=psy_bufs, space="PSUM"))

        qk_sb = per.tile([P, 8, T], f16)             # chunks 0-3: Q^T, 4-7: K^T
        v_sb = per.tile([P, NTT, HPC, D + 1], f16)   # [kv_p, kv_chunk, head, d|1]
        yt_sb = per.tile([P, 4, T], f16)             # [f%128, f//128, q]
        wo_sb = per.tile([P, 4, 1024], f16)

        # ones column for the softmax denominator ride-along (once, up front)
        nc.vector.memset(v_sb[:, :, :, D:D + 1], 1.0)

        # spread DMA trigger issue (~0.6us each on an engine queue) across
        # the otherwise-idle engines, and keep transfers small enough that
        # the first projection chains start while later slices stream in
        x_all = xpool.tile([P, NE, T], f16, tag="x")
        wv_sb = wpool.tile([P, NE, 512], f16, tag="wv", bufs=1)
        wqk_all = wpool.tile([P, 8, NE, P], f16, tag="wq", bufs=1)
        issuers = [nc.sync, nc.scalar, nc.gpsimd]
        nd = 0

        def dma(dst, srcv):
            nonlocal nd
            issuers[nd % len(issuers)].dma_start(dst, srcv)
            nd += 1

        for th in range(4):
            tsl = slice(th * 512, (th + 1) * 512)
            for e in range(NE):
                dma(x_all[:, e, tsl], xP_d[:, e, tsl])
            if th == 0:
                for e in range(NE):
                    dma(wv_sb[:, e, :], wvP_d[:, e, :])
            if th == 1:
                for ft in range(8):
                    dma(wqk_all[:, ft], wqkP_d[:, ft])
        for o in range(4):
            dma(wo_sb[:, o, :], woP_d[:, o, :])

        def proj_quarter(th):
            tsl = slice(th * 512, (th + 1) * 512)
            # V projection for this quarter: natural layout [T, 512]
            for tti in range(4):
                tt = th * 4 + tti
                ps = psp.tile([P, 512], FP, tag="pp")
                for e in range(NE):
                    nc.tensor.matmul(
                        ps,
                        lhsT=x_all[:, e, tt * P:(tt + 1) * P],
                        rhs=wv_sb[:, e, :],
                        start=(e == 0), stop=(e == NE - 1))
                nc.vector.tensor_copy(
                    v_sb[:, tt, :, 0:D],
                    ps.rearrange("p (h d) -> p h d", h=HPC))

            # QK^T projection for this T-quarter: [f, T] layout
            for ft in (0, 4, 1, 5, 2, 6, 3, 7):
                ps = psp.tile([P, 512], FP, tag="pp")
                for e in range(NE):
                    nc.tensor.matmul(
                        ps,
                        lhsT=wqk_all[:, ft, e, :],
                        rhs=x_all[:, e, tsl],
                        start=(e == 0), stop=(e == NE - 1))
                nc.vector.tensor_copy(qk_sb[:, ft, tsl], ps)

        def att_core(j):
            jsl = slice(j * 512, (j + 1) * 512)
            for c in range(4):  # head pair (2c, 2c+1)
                nkv = 4 * j + 4
                yps = [psy.tile([P, 512], FP, tag="y",
                                name=f"yps{rep}_{c}_{j}_{k}")[0:65]
                       for k in range(2)]

                def emit_pv(i, pv):
                    # the stop matmul must cover the full accumulation width
                    # (HW PSUM group-close semantics), so the final chunk's
                    # PV runs untrimmed; its dead ptt region was zeroed.
                    off = i - 4 * j
                    pq0 = P * off if (off > 0 and i < nkv - 1) else 0
                    for hh in range(2):
                        nc.tensor.matmul(
                            yps[hh][:, pq0:512],
                            lhsT=v_sb[:, i, 2 * c + hh, :],
                            rhs=pv[:, hh, pq0:512],
                            start=(i == 0), stop=(i == nkv - 1),
                            skip_group_check=True)

                # PV trails S by one chunk so the PE does not sit on exp(i)
                prev_pv = None
                for i in range(nkv):
                    off = i - 4 * j
                    q0 = P * off if off > 0 else 0
                    spt = pss.tile([P, 1024], FP, tag="s")
                    for hh in range(2):
                        h0 = hh * 512
                        nc.tensor.matmul(
                            spt[:, h0 + q0:h0 + 512],
                            lhsT=qk_sb[64 * hh:64 * hh + 64, 4 + c,
                                       i * P:(i + 1) * P],
                            rhs=qk_sb[64 * hh:64 * hh + 64, c,
                                      j * 512 + q0:(j + 1) * 512],
                            start=True, stop=True)
                    ptt = ptp.tile([P, 1024], f16, tag="pt")
                    pv = ptt.rearrange("p (h q) -> p h q", h=2)
                    sv = spt.rearrange("p (h q) -> p h q", h=2)
                    if off < 0:
                        nc.scalar.activation(ptt, spt, Exp, scale=float(SCALE))
                    else:
                        # diagonal chunk: exp live columns only, then causal
                        # select of the 128-wide triangle block
                        nc.scalar.activation(pv[:, :, q0:512], sv[:, :, q0:512],
                                             Exp, scale=float(SCALE))
                        if i == nkv - 1:
                            # final chunk feeds the full-width stop matmul:
                            # zero the dead cols (write-only, no stale read)
                            nc.gpsimd.memset(pv[:, :, 0:q0], 0.0)
                        nc.gpsimd.affine_select(
                            pv[:, :, q0:q0 + P], pv[:, :, q0:q0 + P],
                            pattern=[[0, 2], [1, P]],
                            compare_op=IS_GE,
                            fill=0.0, base=0, channel_multiplier=-1)
                    if prev_pv is not None:
                        emit_pv(*prev_pv)
                    prev_pv = (i, pv)
                emit_pv(*prev_pv)

                # normalize: 1/l on DVE (approx-fast), GpSimd broadcast, MULT.
                # Single-row ops run at partition 0 (recip/broadcast misbehave
                # on HW at partition offset 64); ScalarE does the
                # cross-partition PSUM p64 -> SBUF p0 hop.
                lsb = nrm.tile([1, 1024], f32, tag="lg")
                rcb = nrm.tile([1, 1024], f32, tag="rc")
                bc = nrm.tile([P, 1024], f32, tag="bc")
                for hh in range(2):
                    hsl = slice(hh * 512, (hh + 1) * 512)
                    nc.scalar.copy(lsb[0:1, hsl], yps[hh][64:65, :])
                    nc.vector.reciprocal_approx_fast(
                        rcb[0:1, hsl], lsb[0:1, hsl])
                    nc.gpsimd.partition_broadcast(
                        bc[0:64, hsl], rcb[0:1, hsl])
                nc.vector.tensor_tensor(
                    yt_sb[0:64, c, jsl], yps[0][0:64, :], bc[0:64, 0:512], MULT)
                tmp = nrm.tile([64, 512], f16, tag="tmp")
                nc.vector.tensor_tensor(
                    tmp, yps[1][0:64, :], bc[0:64, 512:1024], MULT)
                nc.sync.dma_start(yt_sb[64:128, c, jsl], tmp)
                if dbg_d is not None:
                    nc.sync.dma_start(dbg_d["dbg_l"][j, c, :], lsb[0:1, :])
                    nc.sync.dma_start(dbg_d["dbg_rc"][j, c, :], rcb[0:1, :])
                    nc.sync.dma_start(dbg_d["dbg_bc"][j, c, :, :], bc[0:64, :])

        def out_proj(j):
            for tti in range(4):
                tt = 4 * j + tti
                st = ost.tile([P, 1024], f32, tag="st")
                for half in range(2):
                    ps = psp.tile([P, 512], FP, tag="pp")
                    for c2 in range(4):
                        nc.tensor.matmul(
                            ps,
                            lhsT=yt_sb[:, c2, tt * P:(tt + 1) * P],
                            rhs=wo_sb[:, c2, half * 512:(half + 1) * 512],
                            start=(c2 == 0), stop=(c2 == 3))
                    nc.vector.tensor_copy(st[:, half * 512:(half + 1) * 512], ps)
                nc.sync.dma_start(out_d[tt * P:(tt + 1) * P, :], st)

        # Emission (= scheduler priority) order: attention j0/j1 interleaves
        # with projection of the second T-half; out-projections are deferred
        # so their PSUM slots never gate projection, and they act as PE
        # filler during the Scalar-bound later attention blocks.
        proj_quarter(0)
        att_core(0)
        proj_quarter(1)
        att_core(1)
        proj_quarter(2)
        att_core(2)
        proj_quarter(3)
        if dbg_d is not None:
            nc.sync.dma_start(dbg_d["dbg_qk"][:, :, :], qk_sb)
            nc.sync.dma_start(dbg_d["dbg_v"][:, :, :, :], v_sb)
        out_proj(0)
        out_proj(1)
        att_core(3)
        out_proj(2)
        out_proj(3)
        if dbg_d is not None:
            nc.sync.dma_start(dbg_d["dbg_yt"][:, :, :], yt_sb)


def _shard_inputs(x, w_qkv, w_out):
    in_maps = []
    for core in range(8):
        b, hg = core // 2, core % 2
        sl = slice(hg * 512, (hg + 1) * 512)
        wq = w_qkv[0:1024][sl]
        wk = w_qkv[1024:2048][sl]
        wv = w_qkv[2048:3072][sl]
        # [partition, ...] prepack so every DMA line is large and contiguous
        qkT = np.concatenate([wq, wk], axis=0).T.astype(np.float16)  # [E, 1024]
        wqkP = qkT.reshape(NE, P, 8, P).transpose(1, 2, 0, 3)  # [p, ft, e, f]
        wvP = wv.T.astype(np.float16).reshape(NE, P, 512).transpose(1, 0, 2)
        woP = w_out[:, sl].T.astype(np.float16).reshape(4, P, E).transpose(1, 0, 2)
        xP = x[b].T.astype(np.float16).reshape(NE, P, T).transpose(1, 0, 2)
        in_maps.append({
            "xP": np.ascontiguousarray(xP),
            "wqkP": np.ascontiguousarray(wqkP),
            "wvP": np.ascontiguousarray(wvP),
            "woP": np.ascontiguousarray(woP),
        })
    return in_maps


def kernel(x, w_qkv, w_out, _trace=False):
    x = np.asarray(x, dtype=np.float32)
    w_qkv = np.asarray(w_qkv, dtype=np.float32)
    w_out = np.asarray(w_out, dtype=np.float32)

    if "nc" not in _CACHE:
        _CACHE["nc"] = build()
    nc = _CACHE["nc"]

    in_maps = _shard_inputs(x, w_qkv, w_out)
    res = bass_utils.run_bass_kernel_spmd(
        nc, in_maps, core_ids=list(range(8)), trace=_trace)
    kernel.last_result = res

    out = np.empty((B, T, E), dtype=np.float32)
    for b in range(B):
        out[b] = res.results[2 * b]["out"] + res.results[2 * b + 1]["out"]
    return out


# revision 31
# speedup vs baseline: 1.0148x; 1.0002x over previous
"""Causal self-attention Trainium2 kernel (B=4, T=2048, E=1024, H=16, D=64).

Sharding: 8 cores = batch(4) x head-group(2). Each core computes the full
attention for 8 heads of one batch element plus its half of the output
projection; the host sums the two out-proj partials per batch element.

Dataflow (per core, all matmul operands fp16, PSUM fp32):
  - Host pre-transposes x and the weights so contraction dims land on
    partitions: xT [E,T], wqkvT [E,1536], woT [512,E], all fp16.
  - Projection produces Q^T/K^T in [d,T] layout (head pairs packed into 128
    partitions) and V in natural [T,d] layout with an interleaved ones
    column per head parity: even heads [d|1], odd heads [1|d] so the PV
    output of the odd head can land on PSUM partitions 63..127 (l on 63,
    y on 64..127) while the even head lands on 0..64 (y 0..63, l 64).
  - Attention per (query tile, head pair): S^T chunks [128kv, <=512q],
    causally trimmed on diagonal chunks -> exp on ScalarE (scale=1/sqrt(D),
    fp16 out) -> triangular affine_select on GpSimd for the single
    diagonal 128x128 sub-block -> yT accumulation with lhsT = V_aug.
  - Softmax normalize without ScalarE: copy the denominator rows to SBUF
    (partition-aligned), GpSimd partition_broadcast, one DVE divide per
    head half writing fp16 yt.
  - Out-projection fused per query tile: natural-layout [T,E] partial via
    lhsT = yT slices.
"""

import os
import numpy as np

import concourse.bass as bass
import concourse.bacc as bacc
import concourse.mybir as mybir
import concourse.tile as tile
from concourse import bass_utils

f32 = mybir.dt.float32
f16 = mybir.dt.float16
FP = mybir.dt.float32  # psum dtype

P = 128
B, T, E = 4, 2048, 1024
H, D = 16, 64
HPC = H // 2            # heads per core = 8
NE = E // P             # 8 e-chunks
NTT = T // P            # 16 kv chunks
NQ = T // 512           # 4 query tiles of 512
SCALE = 1.0 / np.sqrt(D)

Exp = mybir.ActivationFunctionType.Exp
MULT = mybir.AluOpType.mult
DIV = mybir.AluOpType.divide
IS_GE = mybir.AluOpType.is_ge

_CACHE = {}


def build(reps=1, dbg=False, **opts):
    nc = bacc.Bacc("TRN2", target_bir_lowering=False, debug=False, num_devices=8)

    # host-prepacked [partition, ...] layouts so every DMA line is >=2KB
    xP_d = nc.dram_tensor("xP", [P, NE, T], f16, kind="ExternalInput")
    wqkP_d = nc.dram_tensor("wqkP", [P, 8, NE, P], f16, kind="ExternalInput")
    wvP_d = nc.dram_tensor("wvP", [P, NE, 512], f16, kind="ExternalInput")
    woP_d = nc.dram_tensor("woP", [P, 4, E], f16, kind="ExternalInput")
    out_d = nc.dram_tensor("out", [T, E], f32, kind="ExternalOutput")

    dbg_d = None
    if dbg:
        dbg_d = {
            "dbg_qk": nc.dram_tensor("dbg_qk", [P, 8, T], f16,
                                     kind="ExternalOutput"),
            "dbg_v": nc.dram_tensor("dbg_v", [P, NTT, HPC, D + 1], f16,
                                    kind="ExternalOutput"),
            "dbg_l": nc.dram_tensor("dbg_l", [NQ, 4, 1024], f32,
                                    kind="ExternalOutput"),
            "dbg_rc": nc.dram_tensor("dbg_rc", [NQ, 4, 1024], f32,
                                     kind="ExternalOutput"),
            "dbg_bc": nc.dram_tensor("dbg_bc", [NQ, 4, 64, 1024], f32,
                                     kind="ExternalOutput"),
            "dbg_yt": nc.dram_tensor("dbg_yt", [P, 4, T], f16,
                                     kind="ExternalOutput"),
        }

    with tile.TileContext(nc) as tc:
        for rep in range(reps):
            build_body(tc, xP_d, wqkP_d, wvP_d, woP_d, out_d, rep, dbg_d=dbg_d, **opts)
    nc.compile()
    return nc


def build_body(tc, xP_d, wqkP_d, wvP_d, woP_d, out_d, rep=0,
               ptp_bufs=6, pss_bufs=2, psy_bufs=2, dbg_d=None):
    nc = tc.nc

    from contextlib import ExitStack
    with ExitStack() as top:
        per = top.enter_context(tc.tile_pool(name="per", bufs=1))
        wpool = top.enter_context(tc.tile_pool(name="wpool", bufs=2))
        xpool = top.enter_context(tc.tile_pool(name="xpool", bufs=1))
        ptp = top.enter_context(tc.tile_pool(name="ptp", bufs=ptp_bufs))
        nrm = top.enter_context(tc.tile_pool(name="nrm", bufs=2))
        ost = top.enter_context(tc.tile_pool(name="ost", bufs=2))
        # PSUM budget (8 banks): psp 2 (proj + out-proj chains, one tag),
        # pss 2x2 (S chunks), psy 2x1 (PV accumulators)
        psp = top.enter_context(tc.tile_pool(name="psp", bufs=2, space="PSUM"))
        pss = top.enter_context(
            tc.tile_pool(name="pss", bufs=pss_bufs, space="PSUM"))
        psy = top.enter_context(
            tc.tile_pool(name="psy", bufs=psy_bufs, space="PSUM"))

        qk_sb = per.tile([P, 8, T], f16)             # chunks 0-3: Q^T, 4-7: K^T
        v_sb = per.tile([P, NTT, HPC, D + 1], f16)   # [kv_p, kv_chunk, head, d|1]
        yt_sb = per.tile([P, 4, T], f16)             # [f%128, f//128, q]
        wo_sb = per.tile([P, 4, 1024], f16)

        # ones column for the softmax denominator ride-along (once, up front)
        nc.vector.memset(v_sb[:, :, :, D:D + 1], 1.0)

        # spread DMA trigger issue (~0.6us each on an engine queue) across
        # the otherwise-idle engines, and keep transfers small enough that
        # the first projection chains start while later slices stream in
        x_all = xpool.tile([P, NE, T], f16, tag="x")
        wv_sb = wpool.tile([P, NE, 512], f16, tag="wv", bufs=1)
        wqk_all = wpool.tile([P, 8, NE, P], f16, tag="wq", bufs=1)
        issuers = [nc.sync, nc.scalar, nc.gpsimd]
        nd = 0

        def dma(dst, srcv):
            nonlocal nd
            issuers[nd % len(issuers)].dma_start(dst, srcv)
            nd += 1

        for th in range(4):
            tsl = slice(th * 512, (th + 1) * 512)
            for e in range(NE):
                if th == 0:
                    # first V chain only needs cols [0:128]; split so it
                    # starts as soon as the small slice lands
                    dma(x_all[:, e, 0:128], xP_d[:, e, 0:128])
                    dma(x_all[:, e, 128:512], xP_d[:, e, 128:512])
                else:
                    dma(x_all[:, e, tsl], xP_d[:, e, tsl])
            if th == 0:
                for e in range(NE):
                    dma(wv_sb[:, e, :], wvP_d[:, e, :])
            if th == 1:
                for ft in range(8):
                    dma(wqk_all[:, ft], wqkP_d[:, ft])
        for o in range(4):
            dma(wo_sb[:, o, :], woP_d[:, o, :])

        def proj_quarter(th):
            tsl = slice(th * 512, (th + 1) * 512)
            # V projection for this quarter: natural layout [T, 512]
            for tti in range(4):
                tt = th * 4 + tti
                ps = psp.tile([P, 512], FP, tag="pp")
                for e in range(NE):
                    nc.tensor.matmul(
                        ps,
                        lhsT=x_all[:, e, tt * P:(tt + 1) * P],
                        rhs=wv_sb[:, e, :],
                        start=(e == 0), stop=(e == NE - 1))
                nc.vector.tensor_copy(
                    v_sb[:, tt, :, 0:D],
                    ps.rearrange("p (h d) -> p h d", h=HPC))

            # QK^T projection for this T-quarter: [f, T] layout
            for ft in (0, 4, 1, 5, 2, 6, 3, 7):
                ps = psp.tile([P, 512], FP, tag="pp")
                for e in range(NE):
                    nc.tensor.matmul(
                        ps,
                        lhsT=wqk_all[:, ft, e, :],
                        rhs=x_all[:, e, tsl],
                        start=(e == 0), stop=(e == NE - 1))
                nc.vector.tensor_copy(qk_sb[:, ft, tsl], ps)

        def att_core(j):
            jsl = slice(j * 512, (j + 1) * 512)
            for c in range(4):  # head pair (2c, 2c+1)
                nkv = 4 * j + 4
                yps = [psy.tile([P, 512], FP, tag="y",
                                name=f"yps{rep}_{c}_{j}_{k}")[0:65]
                       for k in range(2)]

                def emit_pv(i, pv):
                    # the stop matmul must cover the full accumulation width
                    # (HW PSUM group-close semantics), so the final chunk's
                    # PV runs untrimmed; its dead ptt region was zeroed.
                    off = i - 4 * j
                    pq0 = P * off if (off > 0 and i < nkv - 1) else 0
                    for hh in range(2):
                        nc.tensor.matmul(
                            yps[hh][:, pq0:512],
                            lhsT=v_sb[:, i, 2 * c + hh, :],
                            rhs=pv[:, hh, pq0:512],
                            start=(i == 0), stop=(i == nkv - 1),
                            skip_group_check=True)

                # PV trails S by one chunk so the PE does not sit on exp(i)
                prev_pv = None
                for i in range(nkv):
                    off = i - 4 * j
                    q0 = P * off if off > 0 else 0
                    spt = pss.tile([P, 1024], FP, tag="s")
                    for hh in range(2):
                        h0 = hh * 512
                        nc.tensor.matmul(
                            spt[:, h0 + q0:h0 + 512],
                            lhsT=qk_sb[64 * hh:64 * hh + 64, 4 + c,
                                       i * P:(i + 1) * P],
                            rhs=qk_sb[64 * hh:64 * hh + 64, c,
                                      j * 512 + q0:(j + 1) * 512],
                            start=True, stop=True)
                    ptt = ptp.tile([P, 1024], f16, tag="pt")
                    pv = ptt.rearrange("p (h q) -> p h q", h=2)
                    sv = spt.rearrange("p (h q) -> p h q", h=2)
                    if off < 0:
                        nc.scalar.activation(ptt, spt, Exp, scale=float(SCALE))
                    else:
                        # diagonal chunk: exp live columns only, then causal
                        # select of the 128-wide triangle block
                        nc.scalar.activation(pv[:, :, q0:512], sv[:, :, q0:512],
                                             Exp, scale=float(SCALE))
                        if i == nkv - 1:
                            # final chunk feeds the full-width stop matmul:
                            # zero the dead cols (write-only, no stale read)
                            nc.gpsimd.memset(pv[:, :, 0:q0], 0.0)
                        nc.gpsimd.affine_select(
                            pv[:, :, q0:q0 + P], pv[:, :, q0:q0 + P],
                            pattern=[[0, 2], [1, P]],
                            compare_op=IS_GE,
                            fill=0.0, base=0, channel_multiplier=-1)
                    if prev_pv is not None:
                        emit_pv(*prev_pv)
                    prev_pv = (i, pv)
                emit_pv(*prev_pv)

                # normalize: 1/l on DVE (approx-fast), GpSimd broadcast, MULT.
                # Single-row ops run at partition 0 (recip/broadcast misbehave
                # on HW at partition offset 64); ScalarE does the
                # cross-partition PSUM p64 -> SBUF p0 hop.
                lsb = nrm.tile([1, 1024], f32, tag="lg")
                rcb = nrm.tile([1, 1024], f32, tag="rc")
                bc = nrm.tile([P, 1024], f32, tag="bc")
                for hh in range(2):
                    hsl = slice(hh * 512, (hh + 1) * 512)
                    nc.scalar.copy(lsb[0:1, hsl], yps[hh][64:65, :])
                    nc.vector.reciprocal_approx_fast(
                        rcb[0:1, hsl], lsb[0:1, hsl])
                    nc.gpsimd.partition_broadcast(
                        bc[0:64, hsl], rcb[0:1, hsl])
                nc.vector.tensor_tensor(
                    yt_sb[0:64, c, jsl], yps[0][0:64, :], bc[0:64, 0:512], MULT)
                tmp = nrm.tile([64, 512], f16, tag="tmp")
                nc.vector.tensor_tensor(
                    tmp, yps[1][0:64, :], bc[0:64, 512:1024], MULT)
                nc.sync.dma_start(yt_sb[64:128, c, jsl], tmp)
                if dbg_d is not None:
                    nc.sync.dma_start(dbg_d["dbg_l"][j, c, :], lsb[0:1, :])
                    nc.sync.dma_start(dbg_d["dbg_rc"][j, c, :], rcb[0:1, :])
                    nc.sync.dma_start(dbg_d["dbg_bc"][j, c, :, :], bc[0:64, :])

        def out_proj(j):
            for tti in range(4):
                tt = 4 * j + tti
                st = ost.tile([P, 1024], f32, tag="st")
                for half in range(2):
                    ps = psp.tile([P, 512], FP, tag="pp")
                    for c2 in range(4):
                        nc.tensor.matmul(
                            ps,
                            lhsT=yt_sb[:, c2, tt * P:(tt + 1) * P],
                            rhs=wo_sb[:, c2, half * 512:(half + 1) * 512],
                            start=(c2 == 0), stop=(c2 == 3))
                    nc.vector.tensor_copy(st[:, half * 512:(half + 1) * 512], ps)
                    nc.sync.dma_start(
                        out_d[tt * P:(tt + 1) * P,
                              half * 512:(half + 1) * 512],
                        st[:, half * 512:(half + 1) * 512])

        # Emission (= scheduler priority) order: attention j0/j1 interleaves
        # with projection of the second T-half; out-projections are deferred
        # so their PSUM slots never gate projection, and they act as PE
        # filler during the Scalar-bound later attention blocks.
        proj_quarter(0)
        att_core(0)
        proj_quarter(1)
        att_core(1)
        proj_quarter(2)
        att_core(2)
        proj_quarter(3)
        if dbg_d is not None:
            nc.sync.dma_start(dbg_d["dbg_qk"][:, :, :], qk_sb)
            nc.sync.dma_start(dbg_d["dbg_v"][:, :, :, :], v_sb)
        out_proj(0)
        out_proj(1)
        att_core(3)
        out_proj(2)
        out_proj(3)
        if dbg_d is not None:
            nc.sync.dma_start(dbg_d["dbg_yt"][:, :, :], yt_sb)


def _shard_inputs(x, w_qkv, w_out):
    in_maps = []
    for core in range(8):
        b, hg = core // 2, core % 2
        sl = slice(hg * 512, (hg + 1) * 512)
        wq = w_qkv[0:1024][sl]
        wk = w_qkv[1024:2048][sl]
        wv = w_qkv[2048:3072][sl]
        # [partition, ...] prepack so every DMA line is large and contiguous
        qkT = np.concatenate([wq, wk], axis=0).T.astype(np.float16)  # [E, 1024]
        wqkP = qkT.reshape(NE, P, 8, P).transpose(1, 2, 0, 3)  # [p, ft, e, f]
        wvP = wv.T.astype(np.float16).reshape(NE, P, 512).transpose(1, 0, 2)
        woP = w_out[:, sl].T.astype(np.float16).reshape(4, P, E).transpose(1, 0, 2)
        xP = x[b].T.astype(np.float16).reshape(NE, P, T).transpose(1, 0, 2)
        in_maps.append({
            "xP": np.ascontiguousarray(xP),
            "wqkP": np.ascontiguousarray(wqkP),
            "wvP": np.ascontiguousarray(wvP),
            "woP": np.ascontiguousarray(woP),
        })
    return in_maps


def kernel(x, w_qkv, w_out, _trace=False):
    x = np.asarray(x, dtype=np.float32)
    w_qkv = np.asarray(w_qkv, dtype=np.float32)
    w_out = np.asarray(w_out, dtype=np.float32)

    if "nc" not in _CACHE:
        _CACHE["nc"] = build()
    nc = _CACHE["nc"]

    in_maps = _shard_inputs(x, w_qkv, w_out)
    res = bass_utils.run_bass_kernel_spmd(
        nc, in_maps, core_ids=list(range(8)), trace=_trace)
    kernel.last_result = res

    out = np.empty((B, T, E), dtype=np.float32)
    for b in range(B):
        out[b] = res.results[2 * b]["out"] + res.results[2 * b + 1]["out"]
    return out
